# revision 17
# baseline (speedup 1.0000x reference)
"""Scaled dot-product attention (mask + attn_weights output) on 8 trn2 cores.

Problem: B=2, H=16, S=2048, DK=DV=64, f32.
  scores = Q@K^T/8, masked (mask==0 -> -1e30), softmax, out = attn @ V.
  Returns (output [B,H,S,DV], attn_weights [B,H,S,S]).

Sharding: B*H = 32 heads -> 4 heads per core (contiguous). Cores 0..3 take
batch 0, cores 4..7 batch 1, so each core needs exactly one [S,S] mask.

Per-core per-(q-tile, head) pipeline (q-tile = 128 queries):
  PE    : maskneg -> PSUM via identity matmul (head 0 only; start=True sets
          has_written so later matmuls accumulate), QK fp32r matmuls as a
          delta vs the previous head (so the mask preload is reused), bf16
          transposes of E, bf16 PV matmuls
  ACT   : E = exp(0.125 * (maskneg + QK)) PSUM->SBUF with accum_out row sums
  DVE   : E^T PSUM->SBUF copies (cast to bf16), reciprocal, out-tile scale
  GPSIMD: mask int32 -> maskneg bf16 affine, E->bf16 cast, attn = E * recip
  DMA   : attn tiles out (1 MB each), mask tiles in, Q/K/V in, out tiles out
"""

import sys

import numpy as np

sys.path.insert(0, "/opt/trn_rl_repo")

import concourse.bacc as bacc
import concourse.tile as tile
from concourse import mybir
from concourse.bass_utils import run_bass_kernel_spmd

B, H, S, DK, DV = 2, 16, 2048, 64, 64
HEADS_PER_CORE = 4
N_CORES = 8
QT = 128                   # q-tile rows
NQT = S // QT              # 16 q-tiles
KC = 512                   # k chunk for QK matmul (one PSUM bank)
NKC = S // KC              # 4
NTC = S // 128             # 16 transpose chunks
NEG = -1.0e30
F32 = mybir.dt.float32
F32R = mybir.dt.float32r
BF16 = mybir.dt.bfloat16
I32 = mybir.dt.int32
Copy = mybir.ActivationFunctionType.Copy
Exp = mybir.ActivationFunctionType.Exp

_CACHE = {}


def _build():
    nc = bacc.Bacc("TRN2", target_bir_lowering=False, debug=False)

    q_d = nc.dram_tensor("q", [HEADS_PER_CORE, S, DK], F32, kind="ExternalInput")
    k_d = nc.dram_tensor("k", [HEADS_PER_CORE, S, DK], F32, kind="ExternalInput")
    v_d = nc.dram_tensor("v", [HEADS_PER_CORE, S, DV], F32, kind="ExternalInput")
    m_d = nc.dram_tensor("mask", [S, S], I32, kind="ExternalInput")
    id_d = nc.dram_tensor("ident_in", [128, 128], F32, kind="ExternalInput")
    o_d = nc.dram_tensor("out", [HEADS_PER_CORE, S, DV], F32, kind="ExternalOutput")
    a_d = nc.dram_tensor("attn", [HEADS_PER_CORE, S, S], F32, kind="ExternalOutput")

    with tile.TileContext(nc) as tc:
        with (
            tc.tile_pool(name="persist", bufs=1) as persist,
            tc.tile_pool(name="stage", bufs=2) as stage,
            tc.tile_pool(name="mask32", bufs=2) as mask32p,
            tc.tile_pool(name="maskbf", bufs=2) as maskbfp,
            tc.tile_pool(name="ework", bufs=2) as ework,
            tc.tile_pool(name="attnp", bufs=2) as attnp,
            tc.tile_pool(name="outp", bufs=3) as outp,
            tc.tile_pool(name="small", bufs=4) as small,
            tc.tile_pool(name="ps_s", bufs=1, space="PSUM") as ps_s,
            tc.tile_pool(name="ps_t", bufs=1, space="PSUM") as ps_t,
            tc.tile_pool(name="ps_o", bufs=2, space="PSUM") as ps_o,
        ):
            ident_f32 = persist.tile([128, 128], F32, tag="identf")
            nc.sync.dma_start(out=ident_f32, in_=id_d[:, :])
            ident = persist.tile([128, 128], BF16, tag="ident")
            nc.vector.tensor_copy(out=ident, in_=ident_f32)

            # ---- load + transpose Q/K per head; build stacked pair tiles
            # QTcat_h = [Q_h^T ; Q_{h-1}^T]   (128 part = 2x64 d, 2048 q)
            # KTcat_h = [K_h^T ; -K_{h-1}^T]
            # so matmul(QTcat_h, KTcat_h) = Q_h K_h^T - Q_{h-1} K_{h-1}^T
            qtcat = [persist.tile([128, S], F32R, name=f"qtcat{h}", tag=f"qtcat{h}")
                     for h in range(HEADS_PER_CORE)]
            ktcat = [persist.tile([128, S], F32R, name=f"ktcat{h}", tag=f"ktcat{h}")
                     for h in range(HEADS_PER_CORE)]
            vbf = [persist.tile([128, NTC, DV], BF16, name=f"vbf{h}", tag=f"vbf{h}")
                   for h in range(HEADS_PER_CORE)]
            osb = [persist.tile([128, NQT, DV], F32, name=f"osb{h}", tag=f"osb{h}")
                   for h in range(HEADS_PER_CORE)]

            for h in range(HEADS_PER_CORE):
                for (src, dstlist, negnext) in (
                    (q_d, qtcat, False), (k_d, ktcat, True)
                ):
                    nat = stage.tile([128, NTC, DK], F32, tag="nat")
                    nc.sync.dma_start(
                        out=nat, in_=src[h].rearrange("(t p) d -> p t d", p=128)
                    )
                    ps_tr = ps_s.tile([64, S], F32, tag="s")
                    for t in range(NTC):
                        nc.tensor.transpose(
                            ps_tr[:, t * 128:(t + 1) * 128], nat[:, t, :], ident_f32
                        )
                    nc.scalar.activation(
                        out=dstlist[h][0:64, :], in_=ps_tr[:, :], func=Copy, scale=1.0
                    )
                    if h == 0:
                        # head 0 has no previous head: zero the lower half
                        # (f32r-rounded zeros via ACT, memset can't write f32r)
                        nc.scalar.activation(
                            out=dstlist[0][64:128, :], in_=ps_tr[:, :], func=Copy,
                            scale=0.0,
                        )
                    hn = h + 1
                    if hn < HEADS_PER_CORE:
                        nc.scalar.activation(
                            out=dstlist[hn][64:128, :], in_=ps_tr[:, :], func=Copy,
                            scale=-1.0 if negnext else 1.0,
                        )
                vnat = stage.tile([128, NTC, DV], F32, tag="nat")
                nc.sync.dma_start(
                    out=vnat, in_=v_d[h].rearrange("(t p) d -> p t d", p=128)
                )
                nc.vector.tensor_copy(out=vbf[h], in_=vnat)

            # ---- main loop
            for i in range(NQT):
                # mask tile: int32 [128, 2048] -> maskneg bf16 (0 -> -1e30)
                m32 = mask32p.tile([128, S], I32, tag="m32")
                nc.gpsimd.dma_start(out=m32, in_=m_d[i * QT:(i + 1) * QT, :])
                mbf = maskbfp.tile([128, S], BF16, tag="mbf")
                nc.vector.tensor_scalar(
                    out=mbf, in0=m32, scalar1=-NEG, scalar2=NEG,
                    op0=mybir.AluOpType.mult, op1=mybir.AluOpType.add,
                )

                psum_s = ps_s.tile([128, S], F32, tag="s")
                for h in range(HEADS_PER_CORE):
                    for c in range(NKC):
                        cs = slice(c * KC, (c + 1) * KC)
                        if h == 0:
                            # mask -> PSUM via identity matmul: sets has_written
                            nc.tensor.matmul(
                                psum_s[:, cs], ident, mbf[:, cs],
                                start=True, stop=False, skip_group_check=True,
                            )
                        # QK delta vs previous head (fp32r, N=512 -> 1 cyc/row)
                        nc.tensor.matmul(
                            psum_s[:, cs],
                            qtcat[h][:, i * QT:(i + 1) * QT],
                            ktcat[h][:, cs],
                            start=False, stop=(h == HEADS_PER_CORE - 1),
                            skip_group_check=True,
                        )
                    # E = exp(0.125 * (maskneg + scores)), row sums for free
                    e_f32 = ework.tile([128, S], F32, tag="e")
                    sums = small.tile([128, 1], F32, tag="sums")
                    nc.scalar.activation(
                        out=e_f32, in_=psum_s, func=Exp, scale=0.125,
                        accum_out=sums,
                    )
                    recip = small.tile([128, 1], F32, tag="recip")
                    nc.vector.reciprocal(out=recip, in_=sums)

                    # bf16 copy of E for the transpose+PV path
                    e_bf = ework.tile([128, S], BF16, tag="ebf")
                    nc.vector.tensor_copy(out=e_bf, in_=e_f32)

                    # E^T chunks via PE transpose (bf16 PSUM, 2 banks)
                    et_sb = ework.tile([128, NTC, 128], BF16, tag="et")
                    psum_t = ps_t.tile([128, NTC * 128], BF16, tag="t")
                    for t in range(NTC):
                        nc.tensor.transpose(
                            psum_t[:, t * 128:(t + 1) * 128],
                            e_bf[:, t * 128:(t + 1) * 128], ident,
                        )
                    nc.vector.tensor_copy(
                        out=et_sb,
                        in_=psum_t.rearrange("p (t c) -> p t c", c=128),
                    )

                    # PV: out[q, v] = sum_t ET_t^T @ V_t  (bf16, fp32 accum)
                    psum_o = ps_o.tile([128, DV], F32, tag="o")
                    for t in range(NTC):
                        nc.tensor.matmul(
                            psum_o, et_sb[:, t, :], vbf[h][:, t, :],
                            start=(t == 0), stop=(t == NTC - 1),
                        )
                    nc.vector.tensor_scalar(
                        out=osb[h][:, i, :], in0=psum_o, scalar1=recip,
                        scalar2=None, op0=mybir.AluOpType.mult,
                    )
                    if i == NQT - 1:
                        nc.scalar.dma_start(
                            out=o_d[h].rearrange("(t p) d -> p t d", p=128),
                            in_=osb[h],
                        )

                    # attn tile: E * recip (f32); DMA in 2-head pairs (2 MB)
                    if h % 2 == 0:
                        attn_sb = attnp.tile([128, 2, S], F32, tag="attn")
                    nc.gpsimd.tensor_scalar(
                        out=attn_sb[:, h % 2, :], in0=e_f32, scalar1=recip,
                        scalar2=None, op0=mybir.AluOpType.mult,
                    )
                    if h % 2 == 1:
                        eng = nc.sync
                        eng.dma_start(
                            out=a_d[h - 1:h + 1, i * QT:(i + 1) * QT, :]
                                .rearrange("h p s -> p h s"),
                            in_=attn_sb,
                        )

    nc.compile()
    return nc


def kernel(Q, K, V, mask):
    if "nc" not in _CACHE:
        _CACHE["nc"] = _build()
    nc = _CACHE["nc"]

    Q = np.asarray(Q, dtype=np.float32).reshape(B * H, S, DK)
    K = np.asarray(K, dtype=np.float32).reshape(B * H, S, DK)
    V = np.asarray(V, dtype=np.float32).reshape(B * H, S, DV)
    mask = np.asarray(mask, dtype=np.int32)
    ident = np.eye(128, dtype=np.float32)

    in_maps = []
    for c in range(N_CORES):
        lo = c * HEADS_PER_CORE
        hi = lo + HEADS_PER_CORE
        b = lo // H
        in_maps.append({
            "q": np.ascontiguousarray(Q[lo:hi]),
            "k": np.ascontiguousarray(K[lo:hi]),
            "v": np.ascontiguousarray(V[lo:hi]),
            "mask": np.ascontiguousarray(mask[b, 0]),
            "ident_in": ident,
        })

    res = run_bass_kernel_spmd(nc, in_maps, core_ids=list(range(N_CORES)))
    out = np.empty((B * H, S, DV), np.float32)
    attn = np.empty((B * H, S, S), np.float32)
    for c in range(N_CORES):
        lo = c * HEADS_PER_CORE
        out[lo:lo + HEADS_PER_CORE] = res.results[c]["out"]
        attn[lo:lo + HEADS_PER_CORE] = res.results[c]["attn"]
    return (out.reshape(B, H, S, DV), attn.reshape(B, H, S, S))
